# revision 36
# baseline (speedup 1.0000x reference)
"""Trainium2 Bass kernel for nn_Conv2d_35742717837647.

Problem: stride-1 VALID 2D conv, 7x7 kernel, single in/out channel, scalar
bias.  Input enc_x [64, 1, 512, 512] f32, weight [1, 1, 7, 7] f32, bias [1]
f32.  Output [64, 1, 506, 506] f32.

Strategy
--------
Data-parallel over batch: 8 images per NeuronCore (8 cores).

On each core the conv runs on the TensorEngine as banded matmuls.  For a
tile of 128 consecutive image rows X [128, 512] and each kernel-column
offset dj in 0..6, build a banded stationary matrix A_dj [128, 122] with
A_dj[m + di, m] = w[di, dj].  Then

    (A_dj^T @ X[:, dj:dj+506])[m, j] = sum_di w[di, dj] * x[m+di, j+dj]

and the 7 matmuls (one per dj) accumulate the full conv for 122 output
rows directly in one PSUM bank.  The band matrices are built on the HOST
from the runtime weights (numpy) and shipped as a replicated input; the
image is pre-cast to bf16 on the host (PE runs bf16 at 1 col/cycle vs 4
for f32; accumulation stays f32 in PSUM).  Bias is added by the Scalar
engine while copying PSUM -> SBUF (output rounded to bf16; tolerance is
2e-2, bf16 rounding costs ~2e-3).

Per image: 4 full tiles at row offsets 0/122/244/366 (outputs 0..487)
plus a shared "tail" tile packing rows 488..511 (24 rows) of 4 images
into 96 partitions with a block-diagonal band (outputs 488..505).

Overhead engineering (TimelineSim: 66.4us baseline -> 57.2us):
 - One DMA per image loads all 4 row-tiles (row offsets 0/122/244/366 are
   a uniform 122-row stride, expressed as an overlapped-window AP) into a
   [128, 4*512] SBUF tile; one DMA per image stores all 4 output tiles
   from a [128, 4*512] bf16 tile.  Cuts HWDGE descriptor-gen count ~4x.
 - Stores are bf16 (halves store bytes; well within tolerance).
 - Input loads and the startup-critical bands constant ride the SP ring,
   output stores the Activation ring, the other constants the GPSIMD
   SWDGE ring, so no sequencer serializes loads against stores.
 - The PE runs warm-up matmuls on a zeroed scratch tile during the
   initial DMA wait so the HAM clock-gate is at 8/8 (2.4 GHz) by the time
   the first real matmul issues, and the PE is never idle at startup.
 - The first image is loaded tile-by-tile (the first matmul needs only
   131KB, not 512KB); the last packed tail is the final store so the
   end-of-kernel drain chain hangs off a small (128KB) DMA.
 - One tile-unit (img 1, tile 3) is computed on the otherwise-idle
   Vector engine as 49 shifted multiply-accumulates, shaving its 1.5us
   off the TensorE critical path.  Compute engines cannot read from an
   arbitrary partition base, so an extra overlapped-window DMA pre-stages
   the 6 row-shifted views it needs.
"""

import os
import numpy as np
import ml_dtypes

import bass_rust
import concourse.bacc as bacc
import concourse.mybir as mybir
import concourse.tile as tile
from concourse.bass_utils import run_bass_kernel_spmd

B, H, W = 64, 512, 512
KH, KW = 7, 7
OH, OW = H - KH + 1, W - KW + 1  # 506, 506
NCORES = 8
PER = B // NCORES  # 8 images per core
TSTRIDE = 122  # full-tile row stride; each tile yields 122 out rows
NT = 4  # full tiles per image
TAIL_R0 = 488  # tail tile: rows 488..511 -> out rows 488..505
TAIL_ROWS = H - TAIL_R0  # 24
TAIL_M = OH - NT * TSTRIDE  # 18
TAIL_PACK = 4  # images packed per tail tile

BF16 = mybir.dt.bfloat16
F32 = mybir.dt.float32

_CACHE = {}
LAST_RESULTS = None


DEFAULT_OPTS = dict(
    n_warm=6,  # warm-up matmuls during startup DMA wait
    x_bufs=4,
    psum_bufs=7,
    o_bufs=4,
    dve_off=True,  # offload one tile-unit (img 1, tile 3) to the idle DVE
    skip_dma=False,  # bench-only: no input loads / output stores (PE isolation)
    skip_pe=False,  # bench-only: no matmuls/activation (DMA isolation)
)

DVE_IMG, DVE_T = 1, 3  # tile-unit computed on DVE instead of the PE

# Measured on HW: SBUF<->HBM transfers only hit the fast DMA path when the
# SBUF side is a dense 128-partition AP with 64B-aligned per-partition
# bytes.  So the device writes output in a tile-strided padded layout
# ([imgs, 128, 4*512] + packed tails [2, 128, 512]) and the host slices
# out the valid rows/cols.


def _img_load_ap(x_ap, img):
    """Overlapped-window AP: src[p, t, c] = x[img, 122*t + p, c].

    Pairs with a dest AP [128, 4, 512] over a [128, 2048] tile, so one
    dma_start lands all four row-tiles (halos duplicated in-flight).
    """
    w = x_ap[img].copy()
    w.ap = bass_rust.VecI64Pair([[W, 128], [TSTRIDE * W, NT], [1, W]])
    return w


def _tile4_dst_ap(xt):
    d = xt[:, :].copy()
    d.ap = bass_rust.VecI64Pair([[NT * W, 128], [W, NT], [1, W]])
    return d


def _emit(
    tc, x_ap, bands_ap, bandstail_ap, bias_ap, wcols_ap, out_ap, outt_ap, ctx,
    repeats=1, opts=None,
):
    nc = tc.nc
    o = dict(DEFAULT_OPTS, **(opts or {}))
    if o["skip_dma"] or o["skip_pe"]:
        o["dve_off"] = False

    consts = ctx.enter_context(tc.tile_pool(name="consts", bufs=1))

    # PE warm-up: memset a scratch tile on GPSIMD (starts immediately),
    # then issue matmuls on it.  They queue ahead of the real matmuls and
    # run while the first image/band DMAs are in flight, releasing the
    # HAM clock-gate to 8/8 (2.4 GHz) before the first real matmul.
    warm_t = consts.tile([128, W], BF16, tag="warm")
    nc.gpsimd.memset(warm_t[:], 0.0)

    # bands is on the first real matmul's critical path: it goes FIRST on
    # the SP ring (ahead of the image loads).  bias/bandstail are needed
    # later and ride the GPSIMD SWDGE ring.
    bands_t = consts.tile([128, 128 * KW], BF16, tag="bands")
    nc.sync.dma_start(bands_t[:], bands_ap[:, :])
    bias_t = consts.tile([128, 1], F32, tag="bias")
    nc.gpsimd.dma_start(bias_t[:], bias_ap[:, :])
    bandstail_t = consts.tile([128, 128 * KW], BF16, tag="bandstail")
    nc.gpsimd.dma_start(bandstail_t[:], bandstail_ap[:, :])
    wcols_t = consts.tile([128, KH * KW], F32, tag="wcols")
    if o["dve_off"]:
        nc.gpsimd.dma_start(wcols_t[:], wcols_ap[:, :])

    if o["n_warm"]:
        wps_pool = ctx.enter_context(
            tc.tile_pool(name="wps", bufs=1, space="PSUM")
        )
        wps = wps_pool.tile([128, W], F32, tag="wps")
        for _ in range(o["n_warm"]):
            nc.tensor.matmul(
                wps[:, :], warm_t[0:128, 0:128], warm_t[0:128, :],
                start=True, stop=True,
            )

    xt_shared = None
    if o["skip_dma"]:
        xt_shared = consts.tile([128, NT * W], BF16, tag="xshared")
        nc.gpsimd.memset(xt_shared[:], 0.0)
    ot_shared = None
    if o["skip_pe"]:
        ot_shared = consts.tile([128, NT * W], BF16, tag="oshared")
        nc.gpsimd.memset(ot_shared[:], 0.0)

    x_pool = ctx.enter_context(tc.tile_pool(name="x", bufs=o["x_bufs"]))
    xtail_pool = ctx.enter_context(tc.tile_pool(name="xtail", bufs=2))
    if o["dve_off"]:
        # dedicated buffers for the DVE-offloaded image: the DVE chews on
        # its tiles for ~30us, which must not block the x_pool rotation.
        # Compute engines can only address partitions from base 0 (BIR
        # verifier: no arbitrary partition-base access), so the 6 row-
        # shifted views needed by di=1..6 are pre-staged by one extra
        # overlapped-window DMA load into xsh.
        xoff_pool = ctx.enter_context(tc.tile_pool(name="xoff", bufs=1))
        dve_pool = ctx.enter_context(tc.tile_pool(name="dve", bufs=1))
    psum_pool = ctx.enter_context(
        tc.tile_pool(name="psum", bufs=o["psum_bufs"], space="PSUM")
    )
    o_pool = ctx.enter_context(tc.tile_pool(name="o", bufs=o["o_bufs"]))
    otail_pool = ctx.enter_context(tc.tile_pool(name="otail", bufs=2))

    def conv_tile(xt, col0, kp, band, ot, ocol0):
        """7 accumulating banded matmuls + bias-copy into ot[:, ocol0:+512].

        Band matrices live at 128-column stride in `band`, always used with
        128 stationary columns (band columns past the useful M are
        zero-filled on the host, so the extra PSUM rows are just zeros).
        The dj=0 matmul covers the full 512 columns so every PSUM element
        is written; dj=1..6 accumulate into the valid 506.
        """
        if o["skip_pe"]:
            return
        ps = psum_pool.tile([128, W], F32, tag="ps")
        for dj in range(KW):
            hi = W if dj == 0 else OW
            nc.tensor.matmul(
                ps[0:128, 0:hi],
                band[0:kp, 128 * dj : 128 * dj + 128],
                xt[0:kp, col0 + dj : col0 + dj + hi],
                start=(dj == 0),
                stop=(dj == KW - 1),
            )
        nc.scalar.activation(
            ot[:, ocol0 : ocol0 + W], ps[:, :],
            mybir.ActivationFunctionType.Identity, bias=bias_t[:, :],
        )

    def dve_tile(xt, col0, xsh, otv):
        """One tile-unit as 49 shifted MACs on the (otherwise idle) DVE.

        acc[m, j] accumulates w[di,dj] * x[m+di, j+dj]; di=0 reads the main
        tile, di=1..6 read the pre-shifted copies in xsh (all reads start
        at partition 0 — arbitrary partition bases are illegal for compute
        engines).  f32 accumulation, bf16 inputs — matches the PE path's
        accuracy.  Rows >= 122 / cols >= 506 of otv are never written
        (host slices them off).
        """
        acc = dve_pool.tile([128, W], F32, tag="acc")
        first_k = True
        for dj in range(KW):
            for di in range(KH):
                k = dj * KH + di
                if di == 0:
                    src = xt[0:TSTRIDE, col0 + dj : col0 + dj + OW]
                else:
                    c = W * (di - 1) + dj
                    src = xsh[0:TSTRIDE, c : c + OW]
                if first_k:
                    nc.vector.tensor_scalar_mul(
                        acc[0:TSTRIDE, 0:OW], src, wcols_t[0:TSTRIDE, k : k + 1]
                    )
                    first_k = False
                else:
                    nc.vector.scalar_tensor_tensor(
                        acc[0:TSTRIDE, 0:OW],
                        src,
                        wcols_t[0:TSTRIDE, k : k + 1],
                        acc[0:TSTRIDE, 0:OW],
                        mybir.AluOpType.mult,
                        mybir.AluOpType.add,
                    )
        nc.vector.tensor_scalar_add(
            otv[0:TSTRIDE, 0:OW], acc[0:TSTRIDE, 0:OW], bias_t[0:TSTRIDE, :]
        )

    first = True
    for img in [i for _ in range(repeats) for i in range(PER)]:
        off = o["dve_off"] and img == DVE_IMG
        if o["skip_dma"]:
            xt = xt_shared
        else:
            if off:
                xt = xoff_pool.tile([128, NT * W], BF16, tag="xt")
            else:
                xt = x_pool.tile([128, NT * W], BF16, tag="xt")
            if first:
                # split the very first image into per-tile loads: the
                # first matmul only needs tile 0 (131KB, ~2.6us end to
                # end) instead of the whole image (512KB, ~4us)
                for t in range(NT):
                    nc.sync.dma_start(
                        xt[:, W * t : W * (t + 1)],
                        x_ap[img, TSTRIDE * t : TSTRIDE * t + 128, :],
                    )
                first = False
            else:
                nc.sync.dma_start(_tile4_dst_ap(xt), _img_load_ap(x_ap, img))
        ot = o_pool.tile([128, NT * W], BF16, tag="ot")
        for t in range(NT):
            if off and t == DVE_T:
                continue
            conv_tile(xt, W * t, 128, bands_t, ot, W * t)
        if off:
            # one DMA stages the 6 row-shifted copies of the offloaded
            # tile: xsh[p, (di-1)*512 + c] = x[img, 122*DVE_T + di + p, c]
            xsh = xoff_pool.tile([128, (KH - 1) * W], BF16, tag="xsh")
            r0 = TSTRIDE * DVE_T + 1  # rows r0+p+k, k=di-1
            src = x_ap[img, r0 : r0 + 128, :].copy()
            src.ap = bass_rust.VecI64Pair([[W, 128], [W, KH - 1], [1, W]])
            dst = xsh[:, :].copy()
            dst.ap = bass_rust.VecI64Pair([[(KH - 1) * W, 128], [W, KH - 1], [1, W]])
            nc.sync.dma_start(dst, src)
            otv = dve_pool.tile([128, W], BF16, tag="otv")
            dve_tile(xt, W * DVE_T, xsh, otv)
        if not o["skip_dma"]:
            src = ot if not o["skip_pe"] else ot_shared
            if off:
                # the PE-computed tiles store normally; the DVE tile gets
                # its own (late) store into the remaining column block
                nc.scalar.dma_start(
                    out_ap[img][:, 0 : DVE_T * W], src[:, 0 : DVE_T * W]
                )
                nc.scalar.dma_start(
                    out_ap[img][:, DVE_T * W : (DVE_T + 1) * W], otv[:, :]
                )
            else:
                nc.scalar.dma_start(out_ap[img], src[:, :])

        if img % TAIL_PACK == TAIL_PACK - 1:
            i0 = img - (TAIL_PACK - 1)
            kp = TAIL_PACK * TAIL_ROWS  # 96 partitions of packed tail rows
            if o["skip_dma"]:
                xtt = xt_shared
            else:
                xtt = xtail_pool.tile([128, W], BF16, tag="xtt")
                for s in range(TAIL_PACK):
                    nc.sync.dma_start(
                        xtt[TAIL_ROWS * s : TAIL_ROWS * (s + 1), :],
                        x_ap[i0 + s, TAIL_R0:H, :],
                    )
            ott = otail_pool.tile([128, W], BF16, tag="ott")
            conv_tile(xtt, 0, kp, bandstail_t, ott, 0)
            if not o["skip_dma"]:
                src = ott if not o["skip_pe"] else ot_shared
                # SP ring: its DGE chain is ~400ns shorter than Act's and
                # the 2nd tail store is the end-of-kernel critical path
                nc.sync.dma_start(outt_ap[i0 // TAIL_PACK], src[:, 0:W])


def build_nc(repeats=1, opts=None):
    from contextlib import ExitStack

    nc = bacc.Bacc(
        "TRN2", target_bir_lowering=False, debug=False, num_devices=NCORES
    )
    x_ap = nc.dram_tensor("x", [PER, H, W], BF16, kind="ExternalInput").ap()
    bands_ap = nc.dram_tensor(
        "bands", [128, 128 * KW], BF16, kind="ExternalInput"
    ).ap()
    bandstail_ap = nc.dram_tensor(
        "bandstail", [128, 128 * KW], BF16, kind="ExternalInput"
    ).ap()
    bias_ap = nc.dram_tensor("bias", [128, 1], F32, kind="ExternalInput").ap()
    wcols_ap = nc.dram_tensor(
        "wcols", [128, KH * KW], F32, kind="ExternalInput"
    ).ap()
    # Padded tile-strided output: out[img][p, 512*t + c] holds conv row
    # 122*t + p, col c (valid p < 122, c < 506); tails hold rows 488+m for
    # 4 packed images per group.  Host slices the valid region.
    out_ap = nc.dram_tensor(
        "out", [PER, 128, NT * W], BF16, kind="ExternalOutput"
    ).ap()
    outt_ap = nc.dram_tensor(
        "outt", [PER // TAIL_PACK, 128, W], BF16, kind="ExternalOutput"
    ).ap()

    with tile.TileContext(nc) as tc:
        with ExitStack() as ctx:
            _emit(
                tc, x_ap, bands_ap, bandstail_ap, bias_ap, wcols_ap,
                out_ap, outt_ap, ctx, repeats, opts,
            )
    nc.compile()
    return nc


def get_nc():
    if "nc" not in _CACHE:
        _CACHE["nc"] = build_nc()
    return _CACHE["nc"]


def build_inputs(weight, bias):
    """Host-side: band matrices (bf16) + replicated bias column."""
    wb = np.asarray(weight, np.float32).reshape(KH, KW).astype(ml_dtypes.bfloat16)
    m = np.arange(TSTRIDE)
    bands = np.zeros((128, 128 * KW), ml_dtypes.bfloat16)
    for dj in range(KW):
        for di in range(KH):
            bands[m + di, 128 * dj + m] = wb[di, dj]

    mt = np.arange(TAIL_M)
    bandstail = np.zeros((128, 128 * KW), ml_dtypes.bfloat16)
    for dj in range(KW):
        for s in range(TAIL_PACK):
            for di in range(KH):
                bandstail[TAIL_ROWS * s + mt + di, 128 * dj + TAIL_M * s + mt] = wb[
                    di, dj
                ]

    bias_col = np.full((128, 1), np.float32(np.asarray(bias).reshape(())))
    # w[di, dj] broadcast down partitions, column k = dj*KH + di (f32, so
    # the DVE-offloaded tile is at least as accurate as the PE path)
    wcols = np.tile(
        np.asarray(weight, np.float32).reshape(KH, KW).T.reshape(1, KH * KW),
        (128, 1),
    )
    return bands, bandstail, bias_col.astype(np.float32), wcols.astype(np.float32)


def kernel(enc_x, weight, bias):
    global LAST_RESULTS
    nc = get_nc()

    xb = np.asarray(enc_x, np.float32).reshape(B, H, W).astype(ml_dtypes.bfloat16)
    bands, bandstail, bias_col, wcols = build_inputs(weight, bias)
    in_maps = [
        {
            "x": xb[PER * c : PER * (c + 1)],
            "bands": bands,
            "bandstail": bandstail,
            "bias": bias_col,
            "wcols": wcols,
        }
        for c in range(NCORES)
    ]
    res = run_bass_kernel_spmd(
        nc,
        in_maps,
        core_ids=list(range(NCORES)),
        trace=bool(int(os.environ.get("KERNEL_TRACE", "0"))),
    )
    LAST_RESULTS = res
    out = np.empty((B, OH, OW), np.float32)
    for c in range(NCORES):
        # full tiles: out rows 122t+m <- out_dev[img][m, 512t:...]
        main = res.results[c]["out"].reshape(PER, 128, NT, W)
        main = main[:, 0:TSTRIDE, :, 0:OW].transpose(0, 2, 1, 3)
        out[PER * c : PER * (c + 1), 0 : NT * TSTRIDE] = main.reshape(
            PER, NT * TSTRIDE, OW
        )
        # tails: out rows 488+m of image 4g+s <- outt_dev[g, 18s+m]
        tail = res.results[c]["outt"][:, 0 : TAIL_PACK * TAIL_M, 0:OW]
        tail = tail.reshape(PER // TAIL_PACK, TAIL_PACK, TAIL_M, OW)
        out[PER * c : PER * (c + 1), NT * TSTRIDE : OH] = tail.reshape(
            PER, TAIL_M, OW
        )
    return out.reshape(B, 1, OH, OW).astype(np.float32)
